# revision 2
# baseline (speedup 1.0000x reference)
"""Self-contained Trainium2 Bass kernel for a 6-layer GPT forward pass.

Problem: B=16, T=256, E=384, H=6 heads, L=6 layers, V=50257.
Returns (logits [B,T,V] f32, loss scalar) matching the jax reference.

Strategy: data-parallel over batch across 8 NeuronCores (2 sequences per
core). Embedding gather happens on host (trivially memory-bound); each
core runs the full transformer stack + LM head on its 512 tokens. The
softmax-denominator (sum of exp of logits) is computed on device fused
with the head matmul; the scalar CE loss is finalized on host from the
returned logits and denominators.

All matmuls run in bf16 with f32 PSUM accumulation; the residual stream,
layernorm statistics and softmax are f32.
"""

import numpy as np
import ml_dtypes

import concourse.bass as bass
import concourse.tile as tile
from concourse import bacc, mybir
from concourse.bass_utils import run_bass_kernel_spmd

# Model dims (hardcoded per the problem spec)
V, E, H, L, T = 50257, 384, 6, 6, 256
HS = E // H          # 64
B = 16
NCORES = 8
BPC = B // NCORES    # 2 sequences per core
N = BPC * T          # 512 tokens per core
NT = N // 128        # 4 token tiles
EC = E // 128        # 3 chunks of the embedding dim
MID = 4 * E          # 1536
MC = MID // 128      # 12 chunks of the MLP hidden dim
LN_EPS = 1e-5
ATT_SCALE = float(E) ** -0.5   # reference scales by n_embd^-0.5
VCHUNK = 2048
SUB = 512            # psum free-dim width for the head matmul

F32 = mybir.dt.float32
BF16 = mybir.dt.bfloat16
AF = mybir.ActivationFunctionType
ALU = mybir.AluOpType
AX = mybir.AxisListType
BF16_NP = ml_dtypes.bfloat16


def _vchunks():
    c = 0
    while c < V:
        w = min(VCHUNK, V - c)
        yield c, w
        c += w


N_SUBCHUNKS = sum((w + SUB - 1) // SUB for _, w in _vchunks())


def build_kernel(has_bproj, has_b1, has_b2, has_bhead, ln1_aff, ln2_aff, lnf_aff):
    nc = bacc.Bacc("TRN2", target_bir_lowering=False, debug=False)

    x0_d = nc.dram_tensor("x0", [N, E], F32, kind="ExternalInput")
    wq_d = nc.dram_tensor("wq", [L, E, E], BF16, kind="ExternalInput")
    wk_d = nc.dram_tensor("wk", [L, E, E], BF16, kind="ExternalInput")
    wv_d = nc.dram_tensor("wv", [L, E, E], BF16, kind="ExternalInput")
    wp_d = nc.dram_tensor("wp", [L, E, E], BF16, kind="ExternalInput")
    w1_d = nc.dram_tensor("w1", [L, E, MID], BF16, kind="ExternalInput")
    w2_d = nc.dram_tensor("w2", [L, MID, E], BF16, kind="ExternalInput")
    bpj_d = nc.dram_tensor("bpj", [L, E], BF16, kind="ExternalInput") if has_bproj else None
    b1_d = nc.dram_tensor("b1", [L, MID], F32, kind="ExternalInput") if has_b1 else None
    b2_d = nc.dram_tensor("b2", [L, E], BF16, kind="ExternalInput") if has_b2 else None
    g1_d = nc.dram_tensor("g1", [L, E], F32, kind="ExternalInput") if ln1_aff else None
    be1_d = nc.dram_tensor("be1", [L, E], F32, kind="ExternalInput") if ln1_aff else None
    g2_d = nc.dram_tensor("g2", [L, E], F32, kind="ExternalInput") if ln2_aff else None
    be2_d = nc.dram_tensor("be2", [L, E], F32, kind="ExternalInput") if ln2_aff else None
    gf_d = nc.dram_tensor("gf", [1, E], F32, kind="ExternalInput") if lnf_aff else None
    bef_d = nc.dram_tensor("bef", [1, E], F32, kind="ExternalInput") if lnf_aff else None
    wh_d = nc.dram_tensor("wh", [E, V], BF16, kind="ExternalInput")
    bh_d = nc.dram_tensor("bh", [1, V], BF16, kind="ExternalInput") if has_bhead else None
    idm_d = nc.dram_tensor("idm", [128, 128], BF16, kind="ExternalInput")
    msk_d = nc.dram_tensor("msk", [128, 128], F32, kind="ExternalInput")

    logits_d = nc.dram_tensor("logits", [N, V], F32, kind="ExternalOutput")
    sumexp_d = nc.dram_tensor("sumexp", [N, 1], F32, kind="ExternalOutput")

    with tile.TileContext(nc) as tc, \
         tc.tile_pool(name="const", bufs=1) as const, \
         tc.tile_pool(name="xres", bufs=10) as xres, \
         tc.tile_pool(name="wts", bufs=2) as wts, \
         tc.tile_pool(name="acts", bufs=2) as acts, \
         tc.tile_pool(name="qk", bufs=3) as qk, \
         tc.tile_pool(name="vp", bufs=8) as vp, \
         tc.tile_pool(name="lnt", bufs=6) as lnt, \
         tc.tile_pool(name="stats", bufs=16) as stats, \
         tc.tile_pool(name="wei", bufs=4) as weip, \
         tc.tile_pool(name="headw", bufs=2) as headw, \
         tc.tile_pool(name="lt", bufs=8) as ltp, \
         tc.tile_pool(name="es", bufs=2) as esp, \
         tc.tile_pool(name="psA", bufs=4, space="PSUM") as psA, \
         tc.tile_pool(name="psB", bufs=2, space="PSUM") as psB:

        # Constants
        id_bf = const.tile([128, 128], BF16, tag="id")
        nc.sync.dma_start(id_bf, idm_d.ap())
        neg_mask = const.tile([128, 128], F32, tag="msk")
        nc.sync.dma_start(neg_mask, msk_d.ap())
        eps_sb = const.tile([128, 1], F32, tag="eps")
        nc.vector.memset(eps_sb, LN_EPS)
        ones_bf = const.tile([1, 128], BF16, tag="ones")
        nc.vector.memset(ones_bf, 1.0)
        ses = [const.tile([128, N_SUBCHUNKS], F32, tag=f"ses{t}", name=f"ses{t}")
               for t in range(NT)]

        # Residual stream (f32, natural [token, E] layout)
        x_tiles = []
        for t in range(NT):
            xt = xres.tile([128, E], F32, tag="x")
            nc.sync.dma_start(xt, x0_d.ap()[t * 128:(t + 1) * 128, :])
            x_tiles.append(xt)

        def layer_norm(xs, g_bc, b_bc, hT_tag):
            """LN over the free dim of each [128,E] tile; returns the
            normalized activations transposed to [E-chunk part, token free]
            in bf16 (the layout every matmul wants as lhsT/rhs)."""
            hT = acts.tile([128, EC, N], BF16, tag=hT_tag)
            for t in range(NT):
                st = stats.tile([128, 6], F32, tag="bnst")
                nc.vector.bn_stats(st, xs[t])
                mv = stats.tile([128, 2], F32, tag="bnag")
                nc.vector.bn_aggr(mv, st)
                sd = stats.tile([128, 1], F32, tag="sd")
                nc.scalar.activation(sd, mv[:, 1:2], AF.Sqrt, bias=eps_sb)
                rstd = stats.tile([128, 1], F32, tag="rstd")
                nc.vector.reciprocal(rstd, sd)
                h = lnt.tile([128, E], BF16, tag="h")
                nc.vector.tensor_scalar(out=h, in0=xs[t], scalar1=mv[:, 0:1],
                                        scalar2=rstd, op0=ALU.subtract, op1=ALU.mult)
                if g_bc is not None:
                    nc.vector.tensor_mul(h, h, g_bc)
                    nc.vector.tensor_add(h, h, b_bc)
                for c in range(EC):
                    ptr = psB.tile([128, 128], BF16, tag="tr")
                    nc.tensor.transpose(ptr, h[:, c * 128:(c + 1) * 128], id_bf)
                    nc.vector.tensor_copy(hT[:, c, t * 128:(t + 1) * 128], ptr)
            return hT

        def load_bcast(pool, dram_ap, tag):
            g = pool.tile([128, E], F32, tag=tag)
            nc.sync.dma_start(g, dram_ap.partition_broadcast(128))
            return g

        for l in range(L):
            # Stream this layer's weights
            wq_sb = wts.tile([128, EC, E], BF16, tag="wq")
            nc.sync.dma_start(wq_sb, wq_d.ap()[l].rearrange("(i p) o -> p i o", p=128))
            wk_sb = wts.tile([128, EC, E], BF16, tag="wk")
            nc.sync.dma_start(wk_sb, wk_d.ap()[l].rearrange("(i p) o -> p i o", p=128))
            wv_sb = wts.tile([128, EC, E], BF16, tag="wv")
            nc.sync.dma_start(wv_sb, wv_d.ap()[l].rearrange("(i p) o -> p i o", p=128))
            wp_sb = wts.tile([128, EC, E], BF16, tag="wp")
            nc.sync.dma_start(wp_sb, wp_d.ap()[l].rearrange("(i p) o -> p i o", p=128))
            w1_sb = wts.tile([128, EC, MID], BF16, tag="w1")
            nc.sync.dma_start(w1_sb, w1_d.ap()[l].rearrange("(i p) o -> p i o", p=128))
            w2_sb = wts.tile([128, MC, E], BF16, tag="w2")
            nc.sync.dma_start(w2_sb, w2_d.ap()[l].rearrange("(m p) o -> p m o", p=128))
            if has_bproj:
                bpj_sb = wts.tile([1, E], BF16, tag="bpj")
                nc.sync.dma_start(bpj_sb, bpj_d.ap()[l:l + 1, :])
            if has_b1:
                b1_sb = wts.tile([128, MC], F32, tag="b1")
                nc.sync.dma_start(b1_sb, b1_d.ap()[l].rearrange("(c p) -> p c", p=128))
            if has_b2:
                b2_sb = wts.tile([1, E], BF16, tag="b2")
                nc.sync.dma_start(b2_sb, b2_d.ap()[l:l + 1, :])
            g1_bc = load_bcast(wts, g1_d.ap()[l:l + 1, :], "g1") if ln1_aff else None
            be1_bc = load_bcast(wts, be1_d.ap()[l:l + 1, :], "be1") if ln1_aff else None
            g2_bc = load_bcast(wts, g2_d.ap()[l:l + 1, :], "g2") if ln2_aff else None
            be2_bc = load_bcast(wts, be2_d.ap()[l:l + 1, :], "be2") if ln2_aff else None

            # ---- attention ----
            hT = layer_norm(x_tiles, g1_bc, be1_bc, "hT")

            # qT/kT in transposed layout [E_out-chunk part, token free]
            qT = qk.tile([128, EC, N], BF16, tag="qk")
            kT = qk.tile([128, EC, N], BF16, tag="qk2")
            for c in range(EC):
                pq = psA.tile([128, N], F32, tag="mm")
                for i in range(EC):
                    nc.tensor.matmul(pq, wq_sb[:, i, c * 128:(c + 1) * 128], hT[:, i, :],
                                     start=(i == 0), stop=(i == EC - 1))
                nc.scalar.activation(qT[:, c, :], pq, AF.Copy, scale=ATT_SCALE)
                pk = psA.tile([128, N], F32, tag="mm")
                for i in range(EC):
                    nc.tensor.matmul(pk, wk_sb[:, i, c * 128:(c + 1) * 128], hT[:, i, :],
                                     start=(i == 0), stop=(i == EC - 1))
                nc.scalar.copy(kT[:, c, :], pk)

            # v in natural layout [token part, E free] (att matmul lhsT)
            v_tiles = []
            for t in range(NT):
                pv = psA.tile([128, E], F32, tag="mm")
                for i in range(EC):
                    nc.tensor.matmul(pv, hT[:, i, t * 128:(t + 1) * 128], wv_sb[:, i, :],
                                     start=(i == 0), stop=(i == EC - 1))
                vt = vp.tile([128, E], BF16, tag="v")
                nc.scalar.copy(vt, pv)
                v_tiles.append(vt)

            attT = acts.tile([128, EC, N], BF16, tag="attT")
            for b in range(BPC):
                for h in range(H):
                    hc, hoff = h // 2, (h % 2) * 64
                    for qt in range(T // 128):
                        gq = b * T + qt * 128
                        width = (qt + 1) * 128   # causal strip within the sequence
                        pw = psA.tile([128, width], F32, tag="mm")
                        nc.tensor.matmul(pw,
                                         qT[hoff:hoff + 64, hc, gq:gq + 128],
                                         kT[hoff:hoff + 64, hc, b * T:b * T + width],
                                         start=True, stop=True)
                        # additive -1e30 mask on the diagonal 128x128 block
                        nc.vector.tensor_add(pw[:, width - 128:width],
                                             pw[:, width - 128:width], neg_mask)
                        rs = stats.tile([128, 1], F32, tag="rs")
                        we = weip.tile([128, width], BF16, tag="wei")
                        nc.scalar.activation(we, pw, AF.Exp, accum_out=rs)
                        rr = stats.tile([128, 1], F32, tag="rr")
                        nc.vector.reciprocal(rr, rs)
                        nc.vector.tensor_scalar_mul(we, in0=we, scalar1=rr)
                        pa = psB.tile([64, 128], F32, tag="att")
                        nk = width // 128
                        for kc in range(nk):
                            ptr = psB.tile([128, 128], BF16, tag="tr")
                            nc.tensor.transpose(ptr, we[:, kc * 128:(kc + 1) * 128], id_bf)
                            wT = weip.tile([128, 128], BF16, tag="weiT")
                            nc.vector.tensor_copy(wT, ptr)
                            kt_glob = b * (T // 128) + kc
                            nc.tensor.matmul(pa, v_tiles[kt_glob][:, h * 64:(h + 1) * 64],
                                             wT, start=(kc == 0), stop=(kc == nk - 1))
                        nc.scalar.copy(attT[hoff:hoff + 64, hc, gq:gq + 128], pa)

            # proj + residual
            x1_tiles = []
            for t in range(NT):
                pp = psA.tile([128, E], F32, tag="mm")
                for i in range(EC):
                    nc.tensor.matmul(pp, attT[:, i, t * 128:(t + 1) * 128], wp_sb[:, i, :],
                                     start=(i == 0), stop=(i == EC - 1 and not has_bproj))
                if has_bproj:
                    nc.tensor.matmul(pp, ones_bf[0:1, :], bpj_sb[0:1, :],
                                     start=False, stop=True)
                x1 = xres.tile([128, E], F32, tag="x")
                nc.vector.tensor_add(x1, x_tiles[t], pp)
                x1_tiles.append(x1)

            # ---- MLP ----
            h2T = layer_norm(x1_tiles, g2_bc, be2_bc, "hT")
            midT = acts.tile([128, MC, N], BF16, tag="midT")
            for m in range(MC):
                pm = psA.tile([128, N], F32, tag="mm")
                for i in range(EC):
                    nc.tensor.matmul(pm, w1_sb[:, i, m * 128:(m + 1) * 128], h2T[:, i, :],
                                     start=(i == 0), stop=(i == EC - 1))
                nc.scalar.activation(midT[:, m, :], pm, AF.Relu,
                                     bias=(b1_sb[:, m:m + 1] if has_b1 else 0.0))
            x2_tiles = []
            for t in range(NT):
                p2 = psA.tile([128, E], F32, tag="mm")
                for m in range(MC):
                    nc.tensor.matmul(p2, midT[:, m, t * 128:(t + 1) * 128], w2_sb[:, m, :],
                                     start=(m == 0), stop=(m == MC - 1 and not has_b2))
                if has_b2:
                    nc.tensor.matmul(p2, ones_bf[0:1, :], b2_sb[0:1, :],
                                     start=False, stop=True)
                x2 = xres.tile([128, E], F32, tag="x")
                nc.vector.tensor_add(x2, x1_tiles[t], p2)
                x2_tiles.append(x2)
            x_tiles = x2_tiles

        # ---- final LN + LM head ----
        gf_bc = load_bcast(const, gf_d.ap()[0:1, :], "gf") if lnf_aff else None
        bef_bc = load_bcast(const, bef_d.ap()[0:1, :], "bef") if lnf_aff else None
        xfT = layer_norm(x_tiles, gf_bc, bef_bc, "xfT")

        wh_re = wh_d.ap().rearrange("(i p) v -> p i v", p=128)
        sidx = [0] * NT
        for c0, cw in _vchunks():
            whc = headw.tile([128, EC, cw], BF16, tag="wh")
            nc.sync.dma_start(whc, wh_re[:, :, c0:c0 + cw])
            if has_bhead:
                bhc = headw.tile([1, cw], BF16, tag="bh")
                nc.sync.dma_start(bhc, bh_d.ap()[0:1, c0:c0 + cw])
            for t in range(NT):
                for s0 in range(0, cw, SUB):
                    sw = min(SUB, cw - s0)
                    ph = psA.tile([128, sw], F32, tag="mm")
                    for i in range(EC):
                        nc.tensor.matmul(ph, xfT[:, i, t * 128:(t + 1) * 128],
                                         whc[:, i, s0:s0 + sw],
                                         start=(i == 0), stop=(i == EC - 1 and not has_bhead))
                    if has_bhead:
                        nc.tensor.matmul(ph, ones_bf[0:1, :], bhc[0:1, s0:s0 + sw],
                                         start=False, stop=True)
                    lt = ltp.tile([128, SUB], F32, tag="lt")
                    nc.vector.tensor_copy(lt[:, :sw], ph)
                    nc.sync.dma_start(
                        logits_d.ap()[t * 128:(t + 1) * 128, c0 + s0:c0 + s0 + sw],
                        lt[:, :sw])
                    es = esp.tile([128, SUB], F32, tag="es")
                    nc.scalar.activation(es[:, :sw], ph, AF.Exp,
                                         accum_out=ses[t][:, sidx[t]:sidx[t] + 1])
                    sidx[t] += 1
        for t in range(NT):
            red = stats.tile([128, 1], F32, tag="sered")
            nc.vector.reduce_sum(red, ses[t][:, :sidx[t]], axis=AX.X)
            nc.sync.dma_start(sumexp_d.ap()[t * 128:(t + 1) * 128, :], red)

    nc.compile()
    return nc


_CACHE = {}


def _get_kernel(flags):
    if flags not in _CACHE:
        _CACHE[flags] = build_kernel(*flags)
    return _CACHE[flags]


def kernel(idx, targets, tok_emb, pos_emb, Wq, Wk, Wv, Wproj, bproj,
           W1, b1, W2, b2, ln1_g, ln1_b, ln2_g, ln2_b, lnf_g, lnf_b,
           Whead, bhead):
    idx = np.asarray(idx)
    targets = np.asarray(targets)
    tok_emb = np.asarray(tok_emb, np.float32)
    pos_emb = np.asarray(pos_emb, np.float32)

    flags = (
        bool(np.any(np.asarray(bproj))), bool(np.any(np.asarray(b1))),
        bool(np.any(np.asarray(b2))), bool(np.any(np.asarray(bhead))),
        bool(np.any(np.asarray(ln1_g) != 1) or np.any(np.asarray(ln1_b))),
        bool(np.any(np.asarray(ln2_g) != 1) or np.any(np.asarray(ln2_b))),
        bool(np.any(np.asarray(lnf_g) != 1) or np.any(np.asarray(lnf_b))),
    )
    nc = _get_kernel(flags)

    # Host-side embedding gather (memory-trivial) + shard over cores
    x0 = tok_emb[idx] + pos_emb[:T][None]          # [B, T, E] f32
    x0 = np.ascontiguousarray(x0.reshape(NCORES, N, E), np.float32)

    bf = lambda a: np.ascontiguousarray(np.asarray(a, np.float32)).astype(BF16_NP)
    shared = {
        "wq": bf(Wq), "wk": bf(Wk), "wv": bf(Wv), "wp": bf(Wproj),
        "w1": bf(W1), "w2": bf(W2), "wh": bf(Whead),
        "idm": np.eye(128, dtype=BF16_NP),
        "msk": np.where(np.arange(128)[None, :] <= np.arange(128)[:, None],
                        0.0, -1e30).astype(np.float32),
    }
    if flags[0]:
        shared["bpj"] = bf(bproj)
    if flags[1]:
        shared["b1"] = np.asarray(b1, np.float32)
    if flags[2]:
        shared["b2"] = bf(b2)
    if flags[3]:
        shared["bh"] = bf(bhead).reshape(1, V)
    if flags[4]:
        shared["g1"] = np.asarray(ln1_g, np.float32)
        shared["be1"] = np.asarray(ln1_b, np.float32)
    if flags[5]:
        shared["g2"] = np.asarray(ln2_g, np.float32)
        shared["be2"] = np.asarray(ln2_b, np.float32)
    if flags[6]:
        shared["gf"] = np.asarray(lnf_g, np.float32).reshape(1, E)
        shared["bef"] = np.asarray(lnf_b, np.float32).reshape(1, E)

    in_maps = [dict(shared, x0=x0[c]) for c in range(NCORES)]
    res = run_bass_kernel_spmd(nc, in_maps, core_ids=list(range(NCORES)))

    logits = np.concatenate([r["logits"].reshape(BPC, T, V) for r in res.results], 0)
    sumexp = np.concatenate([r["sumexp"].reshape(BPC, T) for r in res.results], 0)

    tgt = np.take_along_axis(logits.reshape(B * T, V),
                             targets.reshape(B * T, 1).astype(np.int64), axis=1)[:, 0]
    loss = np.float32(np.mean(np.log(sumexp.astype(np.float64)).reshape(-1)
                              - tgt.astype(np.float64)))
    return logits, loss
